# revision 1
# baseline (speedup 1.0000x reference)
"""EnhancedCorrelationGNN Trainium2 kernel (8 NeuronCores, SPMD).

Strategy: destination-sorted edge processing with node-range output sharding.
 - Host (free): counting-sort edges by dst, partition nodes into 8 ranges of
   6272 (49 blocks x 128 nodes per core). Per block, edges are split by src
   half (dma_gather int16 index limit) and padded to 128-edge tiles with
   cross-core-uniform tile counts (one SPMD program).
 - Phase 1 (device): h = x @ W plus attention projections in ONE matmul per
   128-node tile (rhs = [W@a_dst | W | W@a_src] assembled on-chip), AllGather
   of the [h|attn_s] node table (768B rows). attn_d stays core-local.
 - Phase 2 (device): per 32-tile chunk, dma_gather of [h|as] rows by src and
   ad rows by dst; batched VectorE ops compute leaky-relu scores, ScalarE
   exp, messages; one-hot segment matrix via is_equal(dst_local, iota);
   per-tile TensorE matmul scatter-accumulates [msgs | p] into the block
   PSUM; per block normalize by 1/(sum p + 1e-10), add bias, DMA out.
 - No AllReduce: softmax denominators and sums stay core-local because
   output is sharded by destination node range.
"""
import sys

if "/opt/trn_rl_repo" not in sys.path:
    sys.path.insert(0, "/opt/trn_rl_repo")

import numpy as np

import concourse.bass as bass
import concourse.bacc as bacc
import concourse.mybir as mybir
import concourse.tile as tile
from concourse.bass_utils import run_bass_kernel_spmd

# ---------------------------------------------------------------- constants
N = 50000
E = 800000
IN_F = 128
H = 8
HD = 16
OUT_F = H * HD          # 128
ALPHA = 0.2
EPS = 1e-10

NCORES = 8
P = 128
NPC = 6272              # nodes per core = 49 * 128; 8*6272 = 50176 >= N
NPAD = NCORES * NPC     # 50176
NBLK = NPC // P         # 49
HALF = NPAD // 2        # 25088 (aligned to core boundary: cores 0-3 / 4-7)

ROW = 192               # table row floats: h(128) | as(8) | pad(56) -> 768B
AS_OFF = 128            # attn_s offset within row
ADROW = 64              # ad table row floats: ad(8) | pad(56) -> 256B
CHUNK_TILES = 16        # tiles per gather/DVE chunk
IDX_COLS = CHUNK_TILES * P // 16   # wrapped int16 idx columns per chunk
PAD_DSTL = 300.0        # one-hot miss sentinel (matches no iota value)

FP = mybir.dt.float32


# ---------------------------------------------------------------- planning
def _cdiv(a, b):
    return -(-a // b)


def _wrap_idx(idx_flat: np.ndarray) -> np.ndarray:
    """[n] -> [128, IDX_COLS] int16: idx j at [j%16, j//16], replicated x8."""
    n = idx_flat.shape[0]
    assert n % 16 == 0
    w = idx_flat.reshape(n // 16, 16).T.astype(np.int16)      # [16, n/16]
    w = np.tile(w, (8, 1))                                    # [128, n/16]
    out = np.zeros((P, IDX_COLS), dtype=np.int16)
    out[:, : w.shape[1]] = w
    return out


def plan_and_inputs(edge_index, edge_weight):
    """Host-side edge partitioning. Returns (plan, per_core_arrays).

    plan (core-independent, defines the SPMD program):
      KA, KB: [NBLK] tiles per (block, half)
      chunks: list of dicts(stream, g0, nt) over stream-major tile ids
      block_tiles: per block, list of (chunk_id, slot) in matmul order
      T, n_chunks
    per_core_arrays[c]:
      src_idx [n_chunks,128,IDX_COLS] i16 (half-relative)
      dst_idx [n_chunks,128,IDX_COLS] i16 (core-relative)
      dstl    [128, T] f32 (block-relative dst, PAD_DSTL for pad slots)
      ew      [128, T] f32
    """
    src = np.asarray(edge_index[0], dtype=np.int64)
    dst = np.asarray(edge_index[1], dtype=np.int64)
    ew = np.asarray(edge_weight, dtype=np.float32)

    order = np.argsort(dst, kind="stable")
    src_s, dst_s, ew_s = src[order], dst[order], ew[order]

    # per (core, block, half) edge index lists (into the sorted arrays)
    cnt = np.zeros((NCORES, NBLK, 2), dtype=np.int64)
    lists = [[[None, None] for _ in range(NBLK)] for _ in range(NCORES)]
    # block boundaries over sorted dst
    blk_starts = np.searchsorted(dst_s, np.arange(0, NPAD + 1, P))
    for c in range(NCORES):
        for b in range(NBLK):
            g = c * NBLK + b
            lo, hi = blk_starts[g], blk_starts[g + 1]
            s = src_s[lo:hi]
            mA = s < HALF
            idxs = np.arange(lo, hi)
            lists[c][b][0] = idxs[mA]
            lists[c][b][1] = idxs[~mA]
            cnt[c, b, 0] = mA.sum()
            cnt[c, b, 1] = (~mA).sum()

    KA = np.maximum(_cdiv(cnt[:, :, 0].max(axis=0), P), 1).astype(np.int64)
    KB = _cdiv(cnt[:, :, 1].max(axis=0), P).astype(np.int64)

    T_A = int(KA.sum())
    T_B = int(KB.sum())
    T = T_A + T_B
    cumKA = np.concatenate([[0], np.cumsum(KA)])
    cumKB = np.concatenate([[0], np.cumsum(KB)])

    # chunks: stream-major [0,T_A) then [T_A,T)
    chunks = []
    g = 0
    while g < T_A:
        nt = min(CHUNK_TILES, T_A - g)
        chunks.append(dict(stream=0, g0=g, nt=nt))
        g += nt
    while g < T:
        nt = min(CHUNK_TILES, T - g)
        chunks.append(dict(stream=1, g0=g, nt=nt))
        g += nt
    n_chunks = len(chunks)

    def tile_to_chunk(gidx):
        for ci, ch in enumerate(chunks):
            if ch["g0"] <= gidx < ch["g0"] + ch["nt"]:
                return ci, gidx - ch["g0"]
        raise AssertionError(gidx)

    # precompute chunk lookup as arrays for speed
    chunk_of = np.empty(T, dtype=np.int64)
    slot_of = np.empty(T, dtype=np.int64)
    for ci, ch in enumerate(chunks):
        chunk_of[ch["g0"]: ch["g0"] + ch["nt"]] = ci
        slot_of[ch["g0"]: ch["g0"] + ch["nt"]] = np.arange(ch["nt"])

    block_tiles = []
    for b in range(NBLK):
        tl = []
        for k in range(KA[b]):
            gidx = cumKA[b] + k
            tl.append((int(chunk_of[gidx]), int(slot_of[gidx])))
        for k in range(KB[b]):
            gidx = T_A + cumKB[b] + k
            tl.append((int(chunk_of[gidx]), int(slot_of[gidx])))
        block_tiles.append(tl)

    plan = dict(KA=KA, KB=KB, T=T, T_A=T_A, chunks=chunks,
                block_tiles=block_tiles, n_chunks=n_chunks)

    # ---------------- per-core slot arrays
    per_core = []
    for c in range(NCORES):
        src_rel = np.zeros((T, P), dtype=np.int16)
        dst_rel = np.zeros((T, P), dtype=np.int16)
        dstl = np.full((T, P), PAD_DSTL, dtype=np.float32)
        eww = np.zeros((T, P), dtype=np.float32)
        for b in range(NBLK):
            for half, K, cum, base in ((0, KA, cumKA, 0),
                                       (1, KB, cumKB, T_A)):
                idxs = lists[c][b][half]
                n = idxs.shape[0]
                g0 = base + cum[b]
                nslots = int(K[b]) * P
                # slot j (tile k=j//P, partition p=j%P) <- edge idxs[j]
                s_loc = np.zeros(nslots, dtype=np.int64)
                d_loc = np.zeros(nslots, dtype=np.int64)
                dl = np.full(nslots, PAD_DSTL, dtype=np.float32)
                w = np.zeros(nslots, dtype=np.float32)
                if n:
                    s_loc[:n] = src_s[idxs] - (HALF if half else 0)
                    d_loc[:n] = dst_s[idxs] - c * NPC
                    dl[:n] = (dst_s[idxs] - (c * NPC + b * P)).astype(
                        np.float32)
                    w[:n] = ew_s[idxs]
                sl2 = s_loc.reshape(int(K[b]), P)
                dl2 = d_loc.reshape(int(K[b]), P)
                dll2 = dl.reshape(int(K[b]), P)
                w2 = w.reshape(int(K[b]), P)
                src_rel[g0: g0 + int(K[b])] = sl2.astype(np.int16)
                dst_rel[g0: g0 + int(K[b])] = dl2.astype(np.int16)
                dstl[g0: g0 + int(K[b])] = dll2
                eww[g0: g0 + int(K[b])] = w2

        src_idx = np.zeros((n_chunks, P, IDX_COLS), dtype=np.int16)
        dst_idx = np.zeros((n_chunks, P, IDX_COLS), dtype=np.int16)
        for ci, ch in enumerate(chunks):
            g0, nt = ch["g0"], ch["nt"]
            # edge slot j = u*128 + p maps to tile g0+u, partition p
            flat_s = src_rel[g0: g0 + nt].reshape(nt * P)
            flat_d = dst_rel[g0: g0 + nt].reshape(nt * P)
            src_idx[ci] = _wrap_idx(flat_s)
            dst_idx[ci] = _wrap_idx(flat_d)

        per_core.append(dict(
            src_idx=src_idx, dst_idx=dst_idx,
            dstl=np.ascontiguousarray(dstl.T),   # [128, T]
            ew=np.ascontiguousarray(eww.T),      # [128, T]
        ))

    return plan, per_core


# build stages for HW bisection: 1=phase1+AG only, 2=+gathers,
# 3=+DVE score/S pipeline, 4=full (default)
BUILD_STAGE = 4
# repeat whole kernel body inside one NEFF (for timing by differencing)
REPS = 1


# ---------------------------------------------------------------- builder
def build(plan):
    n_chunks = plan["n_chunks"]
    chunks = plan["chunks"]
    T = plan["T"]

    nc = bacc.Bacc("TRN2", target_bir_lowering=False, debug=False,
                   num_devices=NCORES, num_swdge_queues=4)
    qctr = [0]

    # inputs
    x_t = nc.dram_tensor("x_t", [P, NPC], FP, kind="ExternalInput")
    w_in = nc.dram_tensor("w_in", [P, IN_F], FP, kind="ExternalInput")
    asrep = nc.dram_tensor("asrep", [P, IN_F], FP, kind="ExternalInput")
    adrep = nc.dram_tensor("adrep", [P, IN_F], FP, kind="ExternalInput")
    epwrep = nc.dram_tensor("epwrep", [P, H], FP, kind="ExternalInput")
    epbrep = nc.dram_tensor("epbrep", [P, H], FP, kind="ExternalInput")
    biasrep = nc.dram_tensor("biasrep", [P, OUT_F], FP, kind="ExternalInput")
    iotarep = nc.dram_tensor("iotarep", [P, P], FP, kind="ExternalInput")
    dstl_in = nc.dram_tensor("dstl_in", [P, T], FP, kind="ExternalInput")
    ew_in = nc.dram_tensor("ew_in", [P, T], FP, kind="ExternalInput")
    srcidx_in = nc.dram_tensor("srcidx_in", [n_chunks, P, IDX_COLS],
                               mybir.dt.int16, kind="ExternalInput")
    dstidx_in = nc.dram_tensor("dstidx_in", [n_chunks, P, IDX_COLS],
                               mybir.dt.int16, kind="ExternalInput")
    out = nc.dram_tensor("out", [NPC, OUT_F], FP, kind="ExternalOutput")

    with tile.TileContext(nc) as tc:
        for _rep in range(REPS):
            with tc.tile_pool(name="dram", bufs=1, space="DRAM") as dram, \
                 tc.tile_pool(name="statics", bufs=1) as statics:

                hs_in = dram.tile([NPC, ROW], FP)
                hs_full = dram.tile([NPAD, ROW], FP, addr_space="Shared")
                ad_pad = dram.tile([NPC, ADROW], FP)

                # ---------------- statics
                iota_sb = statics.tile([P, P], FP)
                nc.sync.dma_start(iota_sb[:], iotarep[:])
                epw_sb = statics.tile([P, H], FP)
                nc.sync.dma_start(epw_sb[:], epwrep[:])
                epb_sb = statics.tile([P, H], FP)
                nc.sync.dma_start(epb_sb[:], epbrep[:])
                bias_sb = statics.tile([P, OUT_F], FP)
                nc.sync.dma_start(bias_sb[:], biasrep[:])

                # ---------------- phase 1: node table
                with tc.tile_pool(name="p1", bufs=1) as p1, \
                     tc.tile_pool(name="p1psum", bufs=4, space="PSUM") as p1ps:
                    w_sb = p1.tile([P, IN_F], FP)
                    nc.sync.dma_start(w_sb[:], w_in[:])
                    as_sb = p1.tile([P, IN_F], FP)
                    nc.sync.dma_start(as_sb[:], asrep[:])
                    ad_sb = p1.tile([P, IN_F], FP)
                    nc.sync.dma_start(ad_sb[:], adrep[:])
                    xt_sb = p1.tile([P, NPC], FP)
                    nc.sync.dma_start(xt_sb[:], x_t[:])

                    # rhs_w = [W@a_dst | W | W@a_src]  [128, 144]
                    rhs_w = p1.tile([P, IN_F + 2 * H], FP)
                    nc.vector.tensor_copy(rhs_w[:, H: H + IN_F], w_sb[:])
                    tmp_d = p1.tile([P, IN_F], FP)
                    nc.vector.tensor_tensor(out=tmp_d[:], in0=w_sb[:],
                                            in1=ad_sb[:],
                                            op=mybir.AluOpType.mult)
                    nc.vector.tensor_reduce(
                        out=rhs_w[:, 0:H],
                        in_=tmp_d[:].rearrange("p (h d) -> p h d", d=HD),
                        axis=mybir.AxisListType.X, op=mybir.AluOpType.add)
                    tmp_s = p1.tile([P, IN_F], FP)
                    nc.vector.tensor_tensor(out=tmp_s[:], in0=w_sb[:],
                                            in1=as_sb[:],
                                            op=mybir.AluOpType.mult)
                    nc.vector.tensor_reduce(
                        out=rhs_w[:, H + IN_F: H + IN_F + H],
                        in_=tmp_s[:].rearrange("p (h d) -> p h d", d=HD),
                        axis=mybir.AxisListType.X, op=mybir.AluOpType.add)

                    hs_slice = p1.tile([P, NBLK * ROW], FP)
                    ad_slice = p1.tile([P, NBLK * ADROW], FP)
                    # zero the pad columns (never computed, but DMA'd/gathered)
                    nc.vector.memset(
                        hs_slice[:].rearrange("p (t r) -> p t r", r=ROW)
                        [:, :, AS_OFF + H: ROW], 0.0)
                    nc.vector.memset(ad_slice[:], 0.0)

                    for t in range(NBLK):
                        hpsum = p1ps.tile([P, IN_F + 2 * H], FP, space="PSUM")
                        nc.tensor.matmul(out=hpsum[:],
                                         lhsT=xt_sb[:, t * P: (t + 1) * P],
                                         rhs=rhs_w[:], start=True, stop=True)
                        # [ad | h | as] -> hs row gets [h|as], ad_slice gets ad
                        nc.vector.tensor_copy(
                            hs_slice[:, t * ROW: t * ROW + IN_F + H],
                            hpsum[:, H: 2 * H + IN_F])
                        nc.scalar.activation(
                            ad_slice[:, t * ADROW: t * ADROW + H],
                            hpsum[:, 0:H],
                            mybir.ActivationFunctionType.Copy)

                    nc.sync.dma_start(
                        hs_in[:].rearrange("(t p) r -> p t r", p=P),
                        hs_slice[:].rearrange("p (t r) -> p t r", r=ROW))
                    nc.sync.dma_start(
                        ad_pad[:].rearrange("(t p) h -> p t h", p=P),
                        ad_slice[:].rearrange("p (t h) -> p t h", h=ADROW))

                nc.gpsimd.collective_compute(
                    "AllGather", mybir.AluOpType.bypass,
                    replica_groups=[list(range(NCORES))],
                    ins=[hs_in[:]], outs=[hs_full[:]],
                )

                # ---------------- phase 2
                with tc.tile_pool(name="meta", bufs=1) as meta, \
                     tc.tile_pool(name="gp", bufs=4) as gp, \
                     tc.tile_pool(name="adp", bufs=4) as adp, \
                     tc.tile_pool(name="sp", bufs=4) as sp, \
                     tc.tile_pool(name="rp", bufs=4) as rp, \
                     tc.tile_pool(name="ep", bufs=2) as ep, \
                     tc.tile_pool(name="ip", bufs=4) as ip, \
                     tc.tile_pool(name="op", bufs=3) as opool, \
                     tc.tile_pool(name="bps", bufs=4, space="PSUM") as bps:

                    dstl_sb = meta.tile([P, T], FP)
                    nc.sync.dma_start(dstl_sb[:], dstl_in[:])
                    ew_sb = meta.tile([P, T], FP)
                    nc.sync.dma_start(ew_sb[:], ew_in[:])
                    sidx_all = meta.tile([P, n_chunks, IDX_COLS],
                                         mybir.dt.int16)
                    nc.sync.dma_start(
                        sidx_all[:],
                        srcidx_in[:].rearrange("c p i -> p c i"))
                    didx_all = meta.tile([P, n_chunks, IDX_COLS],
                                         mybir.dt.int16)
                    nc.sync.dma_start(
                        didx_all[:],
                        dstidx_in[:].rearrange("c p i -> p c i"))

                    chunk_tiles = {}

                    def emit_chunk(ci):
                        ch = chunks[ci]
                        g0, nt = ch["g0"], ch["nt"]
                        nidx = nt * P
                        n16 = nidx // 16
                        if BUILD_STAGE == 1:
                            return

                        sidx = sidx_all[:, ci, :]
                        didx = didx_all[:, ci, :]

                        gbuf = gp.tile([P, CHUNK_TILES, ROW], FP, tag="gbuf")
                        half_ap = (hs_full[0:HALF, :] if ch["stream"] == 0
                                   else hs_full[HALF:NPAD, :])
                        nc.gpsimd.dma_gather(
                            out_ap=gbuf[:, :nt, :], in_ap=half_ap,
                            idxs_ap=sidx[:, :n16],
                            num_idxs=nidx, num_idxs_reg=nidx, elem_size=ROW,
                            single_packet=False, queue_num=qctr[0] % 4)
                        qctr[0] += 1

                        adbuf = adp.tile([P, CHUNK_TILES, ADROW], FP,
                                         tag="adbuf")
                        nc.gpsimd.dma_gather(
                            out_ap=adbuf[:, :nt, :], in_ap=ad_pad[:],
                            idxs_ap=didx[:, :n16],
                            num_idxs=nidx, num_idxs_reg=nidx, elem_size=ADROW,
                            single_packet=False, queue_num=qctr[0] % 4)
                        qctr[0] += 1
                        if BUILD_STAGE == 2:
                            chunk_tiles[ci] = (gbuf, adbuf)
                            return

                        # one-hot S [P, nt, 128]
                        s_t = sp.tile([P, CHUNK_TILES * P], FP, tag="s_t")
                        s_v = s_t[:].rearrange("p (t n) -> p t n", n=P)
                        dstl_v = dstl_sb[:, g0: g0 + nt]
                        nc.vector.tensor_tensor(
                            out=s_v[:, :nt, :],
                            in0=dstl_v.unsqueeze(2).broadcast_to([P, nt, P]),
                            in1=iota_sb[:].unsqueeze(1).broadcast_to(
                                [P, nt, P]),
                            op=mybir.AluOpType.is_equal)

                        # scores e = leaky(as+ad) + ew*epw + epb ; p = exp(e)
                        e0 = ep.tile([P, CHUNK_TILES * H], FP, tag="e0")
                        e0v = e0[:].rearrange("p (t h) -> p t h", h=H)[:, :nt, :]
                        nc.vector.tensor_tensor(
                            out=e0v, in0=gbuf[:, :nt, AS_OFF: AS_OFF + H],
                            in1=adbuf[:, :nt, 0:H], op=mybir.AluOpType.add)
                        e1 = ep.tile([P, CHUNK_TILES * H], FP, tag="e1")
                        e1v = e1[:].rearrange("p (t h) -> p t h", h=H)[:, :nt, :]
                        nc.vector.tensor_scalar_mul(out=e1v, in0=e0v,
                                                    scalar1=ALPHA)
                        e2 = ep.tile([P, CHUNK_TILES * H], FP, tag="e2")
                        e2v = e2[:].rearrange("p (t h) -> p t h", h=H)[:, :nt, :]
                        nc.vector.tensor_tensor(out=e2v, in0=e0v, in1=e1v,
                                                op=mybir.AluOpType.max)
                        # ew*epw + epb
                        e3 = ep.tile([P, CHUNK_TILES * H], FP, tag="e3")
                        e3v = e3[:].rearrange("p (t h) -> p t h", h=H)[:, :nt, :]
                        nc.vector.tensor_tensor(
                            out=e3v,
                            in0=ew_sb[:, g0: g0 + nt].unsqueeze(2).broadcast_to(
                                [P, nt, H]),
                            in1=epw_sb[:].unsqueeze(1).broadcast_to([P, nt, H]),
                            op=mybir.AluOpType.mult)
                        e4 = ep.tile([P, CHUNK_TILES * H], FP, tag="e4")
                        e4v = e4[:].rearrange("p (t h) -> p t h", h=H)[:, :nt, :]
                        nc.vector.tensor_tensor(out=e4v, in0=e3v,
                                                in1=epb_sb[:].unsqueeze(1)
                                                .broadcast_to([P, nt, H]),
                                                op=mybir.AluOpType.add)
                        e5 = ep.tile([P, CHUNK_TILES * H], FP, tag="e5")
                        e5v = e5[:].rearrange("p (t h) -> p t h", h=H)[:, :nt, :]
                        nc.vector.tensor_tensor(out=e5v, in0=e2v, in1=e4v,
                                                op=mybir.AluOpType.add)

                        # rhs tile: [msgs(128) | p(8)] per tile
                        rhs = rp.tile([P, CHUNK_TILES * (OUT_F + H)], FP,
                                      tag="rhs")
                        rhs_v = rhs[:].rearrange("p (t f) -> p t f",
                                                 f=OUT_F + H)
                        nc.scalar.activation(
                            rhs_v[:, :nt, OUT_F: OUT_F + H], e5v,
                            mybir.ActivationFunctionType.Exp)
                        # msgs = h * p (broadcast over head dim)
                        nc.vector.tensor_tensor(
                            out=rhs_v[:, :nt, 0:OUT_F].rearrange(
                                "p t (h d) -> p t h d", d=HD),
                            in0=gbuf[:, :nt, 0:IN_F].rearrange(
                                "p t (h d) -> p t h d", d=HD),
                            in1=rhs_v[:, :nt, OUT_F: OUT_F + H].unsqueeze(3)
                                .broadcast_to([P, nt, H, HD]),
                            op=mybir.AluOpType.mult)
                        chunk_tiles[ci] = (s_t, rhs)

                    if BUILD_STAGE < 4:
                        # bisection modes: run phase-2 pieces, dump something
                        for ci in range(n_chunks):
                            emit_chunk(ci)
                        dump = opool.tile([P, OUT_F], FP, tag="dump")
                        if BUILD_STAGE == 1:
                            nc.vector.memset(dump[:], 0.0)
                        elif BUILD_STAGE == 2:
                            g0buf = chunk_tiles[0][0]
                            nc.vector.tensor_copy(dump[:], g0buf[:, 0, 0:OUT_F])
                        else:
                            r0 = chunk_tiles[0][1]
                            nc.vector.tensor_copy(dump[:], r0[:, 0:OUT_F])
                        for b in range(NBLK):
                            nc.sync.dma_start(out[b * P: (b + 1) * P, :],
                                              dump[:])

                    for b in range(NBLK if BUILD_STAGE >= 4 else 0):
                        tl = plan["block_tiles"][b]
                        for (ci, slot) in tl:
                            if ci not in chunk_tiles:
                                emit_chunk(ci)
                        psum_b = bps.tile([P, OUT_F + H], FP, space="PSUM",
                                          tag="psum_b")
                        for i, (ci, slot) in enumerate(tl):
                            s_t, rhs = chunk_tiles[ci]
                            nc.tensor.matmul(
                                out=psum_b[:],
                                lhsT=s_t[:, slot * P: (slot + 1) * P],
                                rhs=rhs[:, slot * (OUT_F + H):
                                        (slot + 1) * (OUT_F + H)],
                                start=(i == 0), stop=(i == len(tl) - 1))
                        # normalize + bias
                        s_eps = opool.tile([P, H], FP, tag="s_eps")
                        nc.vector.tensor_scalar_add(
                            out=s_eps[:], in0=psum_b[:, OUT_F: OUT_F + H],
                            scalar1=EPS)
                        rcp = opool.tile([P, H], FP, tag="rcp")
                        nc.vector.reciprocal(rcp[:], s_eps[:])
                        ob1 = opool.tile([P, OUT_F], FP, tag="ob1")
                        nc.vector.tensor_tensor(
                            out=ob1[:].rearrange("p (h d) -> p h d", d=HD),
                            in0=psum_b[:, 0:OUT_F].rearrange(
                                "p (h d) -> p h d", d=HD),
                            in1=rcp[:].unsqueeze(2).broadcast_to([P, H, HD]),
                            op=mybir.AluOpType.mult)
                        ob2 = opool.tile([P, OUT_F], FP, tag="ob2")
                        nc.vector.tensor_tensor(out=ob2[:], in0=ob1[:],
                                                in1=bias_sb[:],
                                                op=mybir.AluOpType.add)
                        nc.sync.dma_start(out[b * P: (b + 1) * P, :], ob2[:])

    nc.compile()
    # SWDGE constraint: a DMA semaphore may only be updated from one queue.
    # Tile assigns DMASW lanes post-scheduling, so align queue_num to lane.
    for f in nc.m.functions:
        for bb in f.blocks:
            for ins in bb.instructions:
                if type(ins).__name__ == "InstDMAGatherAnt":
                    si = ins.sync_info
                    lane = None
                    for u in si.on_update:
                        nm = u.ant_name or ""
                        if nm.startswith("DMASW"):
                            lane = int(nm[5:].split("_")[0])
                            break
                    assert lane is not None, "gather without DMASW sem"
                    ins.queue_num = lane % 4
    return nc


# ---------------------------------------------------------------- host API
def make_in_maps(x, W, a_src, a_dst, ep_w, ep_b, bias, per_core):
    x = np.asarray(x, dtype=np.float32)
    W = np.asarray(W, dtype=np.float32)
    a_src = np.asarray(a_src, dtype=np.float32)
    a_dst = np.asarray(a_dst, dtype=np.float32)
    ep_w = np.asarray(ep_w, dtype=np.float32)
    ep_b = np.asarray(ep_b, dtype=np.float32)
    bias = np.asarray(bias, dtype=np.float32)

    x_pad = np.zeros((NPAD, IN_F), dtype=np.float32)
    x_pad[:N] = x
    # W [H, IN, HD] -> [IN, H*HD]
    w_flat = np.ascontiguousarray(W.transpose(1, 0, 2).reshape(IN_F, H * HD))
    as_flat = a_src.reshape(H * HD).astype(np.float32)
    ad_flat = a_dst.reshape(H * HD).astype(np.float32)

    rep = lambda v: np.ascontiguousarray(
        np.broadcast_to(v[None, :], (P, v.shape[0])))
    iota = np.broadcast_to(np.arange(P, dtype=np.float32)[None, :], (P, P))

    maps = []
    for c in range(NCORES):
        pc = per_core[c]
        x_t = np.ascontiguousarray(x_pad[c * NPC: (c + 1) * NPC, :].T)
        maps.append({
            "x_t": x_t,
            "w_in": w_flat,
            "asrep": rep(as_flat),
            "adrep": rep(ad_flat),
            "epwrep": rep(ep_w),
            "epbrep": rep(ep_b),
            "biasrep": rep(bias),
            "iotarep": np.ascontiguousarray(iota),
            "dstl_in": pc["dstl"],
            "ew_in": pc["ew"],
            "srcidx_in": pc["src_idx"],
            "dstidx_in": pc["dst_idx"],
        })
    return maps


_CACHE = {}


def kernel(x, edge_index, edge_weight, W, a_src, a_dst, ep_w, ep_b, bias):
    import hashlib
    key = hashlib.sha1(
        np.ascontiguousarray(np.asarray(edge_index, dtype=np.int64))
    ).hexdigest()
    if key not in _CACHE:
        plan, per_core = plan_and_inputs(edge_index, edge_weight)
        nc = build(plan)
        _CACHE[key] = (plan, per_core, nc)
    plan, per_core, nc = _CACHE[key]

    in_maps = make_in_maps(x, W, a_src, a_dst, ep_w, ep_b, bias, per_core)
    res = run_bass_kernel_spmd(nc, in_maps, core_ids=list(range(NCORES)),
                               trace=False)
    out_full = np.empty((NPAD, OUT_F), dtype=np.float32)
    for c in range(NCORES):
        out_full[c * NPC: (c + 1) * NPC] = res.results[c]["out"]
    return out_full[:N]



# revision 7
# speedup vs baseline: 1.3790x; 1.3790x over previous
"""EnhancedCorrelationGNN Trainium2 kernel (8 NeuronCores, SPMD).

Strategy: destination-sorted edge processing with node-range output sharding.
 - Host (free): counting-sort edges by dst, partition nodes into 8 ranges of
   6272 (49 blocks x 128 nodes per core). Per block, edges are split by src
   half (dma_gather int16 index limit) and padded to 128-edge tiles with
   cross-core-uniform tile counts (one SPMD program).
 - Phase 1 (device): h = x @ W plus attention projections in ONE matmul per
   128-node tile (rhs = [W@a_dst | W | W@a_src] assembled on-chip), AllGather
   of the [h|attn_s] node table (768B rows). attn_d stays core-local.
 - Phase 2 (device): per 32-tile chunk, dma_gather of [h|as] rows by src and
   ad rows by dst; batched VectorE ops compute leaky-relu scores, ScalarE
   exp, messages; one-hot segment matrix via is_equal(dst_local, iota);
   per-tile TensorE matmul scatter-accumulates [msgs | p] into the block
   PSUM; per block normalize by 1/(sum p + 1e-10), add bias, DMA out.
 - No AllReduce: softmax denominators and sums stay core-local because
   output is sharded by destination node range.
"""
import sys

if "/opt/trn_rl_repo" not in sys.path:
    sys.path.insert(0, "/opt/trn_rl_repo")

import numpy as np

import concourse.bass as bass
import concourse.bacc as bacc
import concourse.mybir as mybir
import concourse.tile as tile
from concourse.bass_utils import run_bass_kernel_spmd

# ---------------------------------------------------------------- constants
N = 50000
E = 800000
IN_F = 128
H = 8
HD = 16
OUT_F = H * HD          # 128
ALPHA = 0.2
EPS = 1e-10

NCORES = 8
P = 128
NPC = 6272              # nodes per core = 49 * 128; 8*6272 = 50176 >= N
NPAD = NCORES * NPC     # 50176
NBLK = NPC // P         # 49
HALF = NPAD // 2        # 25088 (aligned to core boundary: cores 0-3 / 4-7)

ROW = 192               # table row floats: h(128) | as(8) | pad(56) -> 768B
AS_OFF = 128            # attn_s offset within row
ADROW = 64              # ad table row floats: ad(8) | pad(56) -> 256B
CHUNK_TILES = 16        # tiles per gather/DVE chunk
IDX_COLS = CHUNK_TILES * P // 16   # wrapped int16 idx columns per chunk
PAD_DSTL = 300.0        # one-hot miss sentinel (matches no iota value)

FP = mybir.dt.float32


# ---------------------------------------------------------------- planning
def _cdiv(a, b):
    return -(-a // b)


def _wrap_idx(idx_flat: np.ndarray) -> np.ndarray:
    """[n] -> [128, IDX_COLS] int16: idx j at [j%16, j//16], replicated x8."""
    n = idx_flat.shape[0]
    assert n % 16 == 0
    w = idx_flat.reshape(n // 16, 16).T.astype(np.int16)      # [16, n/16]
    w = np.tile(w, (8, 1))                                    # [128, n/16]
    out = np.zeros((P, IDX_COLS), dtype=np.int16)
    out[:, : w.shape[1]] = w
    return out


def plan_and_inputs(edge_index, edge_weight):
    """Host-side edge partitioning. Returns (plan, per_core_arrays).

    plan (core-independent, defines the SPMD program):
      KA, KB: [NBLK] tiles per (block, half)
      chunks: list of dicts(stream, g0, nt) over stream-major tile ids
      block_tiles: per block, list of (chunk_id, slot) in matmul order
      T, n_chunks
    per_core_arrays[c]:
      src_idx [n_chunks,128,IDX_COLS] i16 (half-relative)
      dst_idx [n_chunks,128,IDX_COLS] i16 (core-relative)
      dstl    [128, T] f32 (block-relative dst, PAD_DSTL for pad slots)
      ew      [128, T] f32
    """
    src = np.asarray(edge_index[0], dtype=np.int64)
    dst = np.asarray(edge_index[1], dtype=np.int64)
    ew = np.asarray(edge_weight, dtype=np.float32)

    order = np.argsort(dst, kind="stable")
    src_s, dst_s, ew_s = src[order], dst[order], ew[order]

    # per (core, block, half) edge index lists (into the sorted arrays)
    cnt = np.zeros((NCORES, NBLK, 2), dtype=np.int64)
    lists = [[[None, None] for _ in range(NBLK)] for _ in range(NCORES)]
    # block boundaries over sorted dst
    blk_starts = np.searchsorted(dst_s, np.arange(0, NPAD + 1, P))
    for c in range(NCORES):
        for b in range(NBLK):
            g = c * NBLK + b
            lo, hi = blk_starts[g], blk_starts[g + 1]
            s = src_s[lo:hi]
            mA = s < HALF
            idxs = np.arange(lo, hi)
            lists[c][b][0] = idxs[mA]
            lists[c][b][1] = idxs[~mA]
            cnt[c, b, 0] = mA.sum()
            cnt[c, b, 1] = (~mA).sum()

    KA = np.maximum(_cdiv(cnt[:, :, 0].max(axis=0), P), 1).astype(np.int64)
    KB = _cdiv(cnt[:, :, 1].max(axis=0), P).astype(np.int64)

    T_A = int(KA.sum())
    T_B = int(KB.sum())
    T = T_A + T_B
    cumKA = np.concatenate([[0], np.cumsum(KA)])
    cumKB = np.concatenate([[0], np.cumsum(KB)])

    # chunks: stream-major [0,T_A) then [T_A,T)
    chunks = []
    g = 0
    while g < T_A:
        nt = min(CHUNK_TILES, T_A - g)
        chunks.append(dict(stream=0, g0=g, nt=nt))
        g += nt
    while g < T:
        nt = min(CHUNK_TILES, T - g)
        chunks.append(dict(stream=1, g0=g, nt=nt))
        g += nt
    n_chunks = len(chunks)

    def tile_to_chunk(gidx):
        for ci, ch in enumerate(chunks):
            if ch["g0"] <= gidx < ch["g0"] + ch["nt"]:
                return ci, gidx - ch["g0"]
        raise AssertionError(gidx)

    # precompute chunk lookup as arrays for speed
    chunk_of = np.empty(T, dtype=np.int64)
    slot_of = np.empty(T, dtype=np.int64)
    for ci, ch in enumerate(chunks):
        chunk_of[ch["g0"]: ch["g0"] + ch["nt"]] = ci
        slot_of[ch["g0"]: ch["g0"] + ch["nt"]] = np.arange(ch["nt"])

    block_tiles = []
    for b in range(NBLK):
        tl = []
        for k in range(KA[b]):
            gidx = cumKA[b] + k
            tl.append((int(chunk_of[gidx]), int(slot_of[gidx])))
        for k in range(KB[b]):
            gidx = T_A + cumKB[b] + k
            tl.append((int(chunk_of[gidx]), int(slot_of[gidx])))
        block_tiles.append(tl)

    plan = dict(KA=KA, KB=KB, T=T, T_A=T_A, chunks=chunks,
                block_tiles=block_tiles, n_chunks=n_chunks)

    # ---------------- per-core slot arrays
    per_core = []
    for c in range(NCORES):
        src_rel = np.zeros((T, P), dtype=np.int16)
        dst_rel = np.zeros((T, P), dtype=np.int16)
        dstl = np.full((T, P), PAD_DSTL, dtype=np.float32)
        eww = np.zeros((T, P), dtype=np.float32)
        for b in range(NBLK):
            for half, K, cum, base in ((0, KA, cumKA, 0),
                                       (1, KB, cumKB, T_A)):
                idxs = lists[c][b][half]
                n = idxs.shape[0]
                g0 = base + cum[b]
                nslots = int(K[b]) * P
                # slot j (tile k=j//P, partition p=j%P) <- edge idxs[j]
                s_loc = np.zeros(nslots, dtype=np.int64)
                d_loc = np.zeros(nslots, dtype=np.int64)
                dl = np.full(nslots, PAD_DSTL, dtype=np.float32)
                w = np.zeros(nslots, dtype=np.float32)
                if n:
                    s_loc[:n] = src_s[idxs] - (HALF if half else 0)
                    d_loc[:n] = dst_s[idxs] - c * NPC
                    dl[:n] = (dst_s[idxs] - (c * NPC + b * P)).astype(
                        np.float32)
                    w[:n] = ew_s[idxs]
                sl2 = s_loc.reshape(int(K[b]), P)
                dl2 = d_loc.reshape(int(K[b]), P)
                dll2 = dl.reshape(int(K[b]), P)
                w2 = w.reshape(int(K[b]), P)
                src_rel[g0: g0 + int(K[b])] = sl2.astype(np.int16)
                dst_rel[g0: g0 + int(K[b])] = dl2.astype(np.int16)
                dstl[g0: g0 + int(K[b])] = dll2
                eww[g0: g0 + int(K[b])] = w2

        src_idx = np.zeros((n_chunks, P, IDX_COLS), dtype=np.int16)
        dst_idx = np.zeros((n_chunks, P, IDX_COLS), dtype=np.int16)
        for ci, ch in enumerate(chunks):
            g0, nt = ch["g0"], ch["nt"]
            # edge slot j = u*128 + p maps to tile g0+u, partition p
            flat_s = src_rel[g0: g0 + nt].reshape(nt * P)
            flat_d = dst_rel[g0: g0 + nt].reshape(nt * P)
            src_idx[ci] = _wrap_idx(flat_s)
            dst_idx[ci] = _wrap_idx(flat_d)

        per_core.append(dict(
            src_idx=src_idx, dst_idx=dst_idx,
            dstl=np.ascontiguousarray(dstl.T),   # [128, T]
            ew=np.ascontiguousarray(eww.T),      # [128, T]
        ))

    return plan, per_core


# build stages for HW bisection: 1=phase1+AG only, 2=+gathers,
# 3=+DVE score/S pipeline, 4=full (default)
BUILD_STAGE = 4
# repeat whole kernel body inside one NEFF (for timing by differencing)
REPS = 1
# stage-2 gather experiments: G_NO_AD skips the ad gather; G_ROW gathers
# only the first G_ROW elems of each hs row (64=256B, 128=512B; 0=full row)
G_NO_AD = False
G_ROW = 0


# ---------------------------------------------------------------- builder
def build(plan):
    n_chunks = plan["n_chunks"]
    chunks = plan["chunks"]
    T = plan["T"]

    nc = bacc.Bacc("TRN2", target_bir_lowering=False, debug=False,
                   num_devices=NCORES, num_swdge_queues=4)
    qctr = [0]

    # inputs
    x_t = nc.dram_tensor("x_t", [P, NPC], FP, kind="ExternalInput")
    w_in = nc.dram_tensor("w_in", [P, IN_F], FP, kind="ExternalInput")
    asrep = nc.dram_tensor("asrep", [P, IN_F], FP, kind="ExternalInput")
    adrep = nc.dram_tensor("adrep", [P, IN_F], FP, kind="ExternalInput")
    epwrep = nc.dram_tensor("epwrep", [P, H], FP, kind="ExternalInput")
    epbrep = nc.dram_tensor("epbrep", [P, H], FP, kind="ExternalInput")
    biasrep = nc.dram_tensor("biasrep", [P, OUT_F], FP, kind="ExternalInput")
    iotarep = nc.dram_tensor("iotarep", [P, P], FP, kind="ExternalInput")
    dstl_in = nc.dram_tensor("dstl_in", [P, T], FP, kind="ExternalInput")
    ew_in = nc.dram_tensor("ew_in", [P, T], FP, kind="ExternalInput")
    srcidx_in = nc.dram_tensor("srcidx_in", [n_chunks, P, IDX_COLS],
                               mybir.dt.int16, kind="ExternalInput")
    dstidx_in = nc.dram_tensor("dstidx_in", [n_chunks, P, IDX_COLS],
                               mybir.dt.int16, kind="ExternalInput")
    out = nc.dram_tensor("out", [NPC, OUT_F], FP, kind="ExternalOutput")

    with tile.TileContext(nc) as tc:
        for _rep in range(REPS):
            with tc.tile_pool(name="dram", bufs=1, space="DRAM") as dram, \
                 tc.tile_pool(name="statics", bufs=1) as statics:

                hs_in = dram.tile([NPC, ROW], FP)
                hs_full = dram.tile([NPAD, ROW], FP, addr_space="Shared")
                ad_pad = dram.tile([NPC, ADROW], FP)

                # ---------------- statics
                iota_sb = statics.tile([P, P], FP)
                nc.sync.dma_start(iota_sb[:], iotarep[:])
                epw_sb = statics.tile([P, H], FP)
                nc.sync.dma_start(epw_sb[:], epwrep[:])
                epb_sb = statics.tile([P, H], FP)
                nc.sync.dma_start(epb_sb[:], epbrep[:])
                bias_sb = statics.tile([P, OUT_F], FP)
                nc.sync.dma_start(bias_sb[:], biasrep[:])

                # ---------------- phase 1: node table
                with tc.tile_pool(name="p1", bufs=1) as p1, \
                     tc.tile_pool(name="p1psum", bufs=4, space="PSUM") as p1ps:
                    w_sb = p1.tile([P, IN_F], FP)
                    nc.sync.dma_start(w_sb[:], w_in[:])
                    as_sb = p1.tile([P, IN_F], FP)
                    nc.sync.dma_start(as_sb[:], asrep[:])
                    ad_sb = p1.tile([P, IN_F], FP)
                    nc.sync.dma_start(ad_sb[:], adrep[:])
                    xt_sb = p1.tile([P, NPC], FP)
                    nc.sync.dma_start(xt_sb[:], x_t[:])

                    # rhs_w = [W@a_dst | W | W@a_src]  [128, 144]
                    rhs_w = p1.tile([P, IN_F + 2 * H], FP)
                    nc.vector.tensor_copy(rhs_w[:, H: H + IN_F], w_sb[:])
                    tmp_d = p1.tile([P, IN_F], FP)
                    nc.vector.tensor_tensor(out=tmp_d[:], in0=w_sb[:],
                                            in1=ad_sb[:],
                                            op=mybir.AluOpType.mult)
                    nc.vector.tensor_reduce(
                        out=rhs_w[:, 0:H],
                        in_=tmp_d[:].rearrange("p (h d) -> p h d", d=HD),
                        axis=mybir.AxisListType.X, op=mybir.AluOpType.add)
                    tmp_s = p1.tile([P, IN_F], FP)
                    nc.vector.tensor_tensor(out=tmp_s[:], in0=w_sb[:],
                                            in1=as_sb[:],
                                            op=mybir.AluOpType.mult)
                    nc.vector.tensor_reduce(
                        out=rhs_w[:, H + IN_F: H + IN_F + H],
                        in_=tmp_s[:].rearrange("p (h d) -> p h d", d=HD),
                        axis=mybir.AxisListType.X, op=mybir.AluOpType.add)

                    hs_slice = p1.tile([P, NBLK * ROW], FP)
                    ad_slice = p1.tile([P, NBLK * ADROW], FP)
                    # zero the pad columns (never computed, but DMA'd/gathered)
                    nc.vector.memset(
                        hs_slice[:].rearrange("p (t r) -> p t r", r=ROW)
                        [:, :, AS_OFF + H: ROW], 0.0)
                    nc.vector.memset(ad_slice[:], 0.0)

                    for t in range(NBLK):
                        hpsum = p1ps.tile([P, IN_F + 2 * H], FP, space="PSUM")
                        nc.tensor.matmul(out=hpsum[:],
                                         lhsT=xt_sb[:, t * P: (t + 1) * P],
                                         rhs=rhs_w[:], start=True, stop=True)
                        # [ad | h | as] -> hs row gets [h|as], ad_slice gets ad
                        nc.vector.tensor_copy(
                            hs_slice[:, t * ROW: t * ROW + IN_F + H],
                            hpsum[:, H: 2 * H + IN_F])
                        nc.scalar.activation(
                            ad_slice[:, t * ADROW: t * ADROW + H],
                            hpsum[:, 0:H],
                            mybir.ActivationFunctionType.Copy)

                    nc.sync.dma_start(
                        hs_in[:].rearrange("(t p) r -> p t r", p=P),
                        hs_slice[:].rearrange("p (t r) -> p t r", r=ROW))
                    nc.sync.dma_start(
                        ad_pad[:].rearrange("(t p) h -> p t h", p=P),
                        ad_slice[:].rearrange("p (t h) -> p t h", h=ADROW))

                nc.gpsimd.collective_compute(
                    "AllGather", mybir.AluOpType.bypass,
                    replica_groups=[list(range(NCORES))],
                    ins=[hs_in[:]], outs=[hs_full[:]],
                )

                # ---------------- phase 2
                with tc.tile_pool(name="meta", bufs=1) as meta, \
                     tc.tile_pool(name="gp", bufs=4) as gp, \
                     tc.tile_pool(name="adp", bufs=4) as adp, \
                     tc.tile_pool(name="sp", bufs=4) as sp, \
                     tc.tile_pool(name="rp", bufs=4) as rp, \
                     tc.tile_pool(name="ep", bufs=2) as ep, \
                     tc.tile_pool(name="ip", bufs=4) as ip, \
                     tc.tile_pool(name="op", bufs=3) as opool, \
                     tc.tile_pool(name="bps", bufs=4, space="PSUM") as bps:

                    dstl_sb = meta.tile([P, T], FP)
                    nc.sync.dma_start(dstl_sb[:], dstl_in[:])
                    ew_sb = meta.tile([P, T], FP)
                    nc.sync.dma_start(ew_sb[:], ew_in[:])
                    sidx_all = meta.tile([P, n_chunks, IDX_COLS],
                                         mybir.dt.int16)
                    nc.sync.dma_start(
                        sidx_all[:],
                        srcidx_in[:].rearrange("c p i -> p c i"))
                    didx_all = meta.tile([P, n_chunks, IDX_COLS],
                                         mybir.dt.int16)
                    nc.sync.dma_start(
                        didx_all[:],
                        dstidx_in[:].rearrange("c p i -> p c i"))

                    chunk_tiles = {}

                    def emit_chunk(ci):
                        ch = chunks[ci]
                        g0, nt = ch["g0"], ch["nt"]
                        nidx = nt * P
                        n16 = nidx // 16
                        if BUILD_STAGE == 1:
                            return

                        sidx = sidx_all[:, ci, :]
                        didx = didx_all[:, ci, :]

                        grow = G_ROW or ROW
                        gbuf = gp.tile([P, CHUNK_TILES, grow], FP, tag="gbuf")
                        half_ap = (hs_full[0:HALF, :] if ch["stream"] == 0
                                   else hs_full[HALF:NPAD, :])
                        if G_ROW:
                            half_ap = half_ap[:, 0:grow]
                        nc.gpsimd.dma_gather(
                            out_ap=gbuf[:, :nt, :], in_ap=half_ap,
                            idxs_ap=sidx[:, :n16],
                            num_idxs=nidx, num_idxs_reg=nidx, elem_size=grow,
                            elem_step=ROW if G_ROW else None,
                            single_packet=False, queue_num=qctr[0] % 4)
                        qctr[0] += 1

                        adbuf = None
                        if not G_NO_AD:
                            adbuf = adp.tile([P, CHUNK_TILES, ADROW], FP,
                                             tag="adbuf")
                            nc.gpsimd.dma_gather(
                                out_ap=adbuf[:, :nt, :], in_ap=ad_pad[:],
                                idxs_ap=didx[:, :n16],
                                num_idxs=nidx, num_idxs_reg=nidx,
                                elem_size=ADROW,
                                single_packet=False, queue_num=qctr[0] % 4)
                            qctr[0] += 1
                        if BUILD_STAGE == 2:
                            chunk_tiles[ci] = (gbuf, adbuf)
                            return

                        # one-hot S [P, nt, 128]
                        s_t = sp.tile([P, CHUNK_TILES * P], FP, tag="s_t")
                        s_v = s_t[:].rearrange("p (t n) -> p t n", n=P)
                        dstl_v = dstl_sb[:, g0: g0 + nt]
                        nc.vector.tensor_tensor(
                            out=s_v[:, :nt, :],
                            in0=dstl_v.unsqueeze(2).broadcast_to([P, nt, P]),
                            in1=iota_sb[:].unsqueeze(1).broadcast_to(
                                [P, nt, P]),
                            op=mybir.AluOpType.is_equal)

                        # scores e = leaky(as+ad) + ew*epw + epb ; p = exp(e)
                        e0 = ep.tile([P, CHUNK_TILES * H], FP, tag="e0")
                        e0v = e0[:].rearrange("p (t h) -> p t h", h=H)[:, :nt, :]
                        nc.vector.tensor_tensor(
                            out=e0v, in0=gbuf[:, :nt, AS_OFF: AS_OFF + H],
                            in1=adbuf[:, :nt, 0:H], op=mybir.AluOpType.add)
                        e1 = ep.tile([P, CHUNK_TILES * H], FP, tag="e1")
                        e1v = e1[:].rearrange("p (t h) -> p t h", h=H)[:, :nt, :]
                        nc.vector.tensor_scalar_mul(out=e1v, in0=e0v,
                                                    scalar1=ALPHA)
                        e2 = ep.tile([P, CHUNK_TILES * H], FP, tag="e2")
                        e2v = e2[:].rearrange("p (t h) -> p t h", h=H)[:, :nt, :]
                        nc.vector.tensor_tensor(out=e2v, in0=e0v, in1=e1v,
                                                op=mybir.AluOpType.max)
                        # ew*epw + epb
                        e3 = ep.tile([P, CHUNK_TILES * H], FP, tag="e3")
                        e3v = e3[:].rearrange("p (t h) -> p t h", h=H)[:, :nt, :]
                        nc.vector.tensor_tensor(
                            out=e3v,
                            in0=ew_sb[:, g0: g0 + nt].unsqueeze(2).broadcast_to(
                                [P, nt, H]),
                            in1=epw_sb[:].unsqueeze(1).broadcast_to([P, nt, H]),
                            op=mybir.AluOpType.mult)
                        e4 = ep.tile([P, CHUNK_TILES * H], FP, tag="e4")
                        e4v = e4[:].rearrange("p (t h) -> p t h", h=H)[:, :nt, :]
                        nc.vector.tensor_tensor(out=e4v, in0=e3v,
                                                in1=epb_sb[:].unsqueeze(1)
                                                .broadcast_to([P, nt, H]),
                                                op=mybir.AluOpType.add)
                        e5 = ep.tile([P, CHUNK_TILES * H], FP, tag="e5")
                        e5v = e5[:].rearrange("p (t h) -> p t h", h=H)[:, :nt, :]
                        nc.vector.tensor_tensor(out=e5v, in0=e2v, in1=e4v,
                                                op=mybir.AluOpType.add)

                        # rhs tile: [msgs(128) | p(8)] per tile
                        rhs = rp.tile([P, CHUNK_TILES * (OUT_F + H)], FP,
                                      tag="rhs")
                        rhs_v = rhs[:].rearrange("p (t f) -> p t f",
                                                 f=OUT_F + H)
                        nc.scalar.activation(
                            rhs_v[:, :nt, OUT_F: OUT_F + H], e5v,
                            mybir.ActivationFunctionType.Exp)
                        # msgs = h * p (broadcast over head dim)
                        nc.vector.tensor_tensor(
                            out=rhs_v[:, :nt, 0:OUT_F].rearrange(
                                "p t (h d) -> p t h d", d=HD),
                            in0=gbuf[:, :nt, 0:IN_F].rearrange(
                                "p t (h d) -> p t h d", d=HD),
                            in1=rhs_v[:, :nt, OUT_F: OUT_F + H].unsqueeze(3)
                                .broadcast_to([P, nt, H, HD]),
                            op=mybir.AluOpType.mult)
                        chunk_tiles[ci] = (s_t, rhs)

                    if BUILD_STAGE < 4:
                        # bisection modes: run phase-2 pieces, dump something
                        for ci in range(n_chunks):
                            emit_chunk(ci)
                        dump = opool.tile([P, OUT_F], FP, tag="dump")
                        if BUILD_STAGE == 1:
                            nc.vector.memset(dump[:], 0.0)
                        elif BUILD_STAGE == 2:
                            g0buf = chunk_tiles[0][0]
                            ncols = min(G_ROW or OUT_F, OUT_F)
                            nc.vector.memset(dump[:], 0.0)
                            nc.vector.tensor_copy(dump[:, 0:ncols],
                                                  g0buf[:, 0, 0:ncols])
                        else:
                            r0 = chunk_tiles[0][1]
                            nc.vector.tensor_copy(dump[:], r0[:, 0:OUT_F])
                        for b in range(NBLK):
                            nc.sync.dma_start(out[b * P: (b + 1) * P, :],
                                              dump[:])

                    for b in range(NBLK if BUILD_STAGE >= 4 else 0):
                        tl = plan["block_tiles"][b]
                        for (ci, slot) in tl:
                            if ci not in chunk_tiles:
                                emit_chunk(ci)
                        psum_b = bps.tile([P, OUT_F + H], FP, space="PSUM",
                                          tag="psum_b")
                        for i, (ci, slot) in enumerate(tl):
                            s_t, rhs = chunk_tiles[ci]
                            nc.tensor.matmul(
                                out=psum_b[:],
                                lhsT=s_t[:, slot * P: (slot + 1) * P],
                                rhs=rhs[:, slot * (OUT_F + H):
                                        (slot + 1) * (OUT_F + H)],
                                start=(i == 0), stop=(i == len(tl) - 1))
                        # normalize + bias
                        s_eps = opool.tile([P, H], FP, tag="s_eps")
                        nc.vector.tensor_scalar_add(
                            out=s_eps[:], in0=psum_b[:, OUT_F: OUT_F + H],
                            scalar1=EPS)
                        rcp = opool.tile([P, H], FP, tag="rcp")
                        nc.vector.reciprocal(rcp[:], s_eps[:])
                        ob1 = opool.tile([P, OUT_F], FP, tag="ob1")
                        nc.vector.tensor_tensor(
                            out=ob1[:].rearrange("p (h d) -> p h d", d=HD),
                            in0=psum_b[:, 0:OUT_F].rearrange(
                                "p (h d) -> p h d", d=HD),
                            in1=rcp[:].unsqueeze(2).broadcast_to([P, H, HD]),
                            op=mybir.AluOpType.mult)
                        ob2 = opool.tile([P, OUT_F], FP, tag="ob2")
                        nc.vector.tensor_tensor(out=ob2[:], in0=ob1[:],
                                                in1=bias_sb[:],
                                                op=mybir.AluOpType.add)
                        nc.sync.dma_start(out[b * P: (b + 1) * P, :], ob2[:])

    nc.compile()
    # SWDGE constraint: a DMA semaphore may only be updated from one queue.
    # Tile assigns DMASW lanes post-scheduling, so align queue_num to lane.
    for f in nc.m.functions:
        for bb in f.blocks:
            for ins in bb.instructions:
                if type(ins).__name__ == "InstDMAGatherAnt":
                    si = ins.sync_info
                    lane = None
                    for u in si.on_update:
                        nm = u.ant_name or ""
                        if nm.startswith("DMASW"):
                            lane = int(nm[5:].split("_")[0])
                            break
                    assert lane is not None, "gather without DMASW sem"
                    ins.queue_num = lane % 4
    return nc


# ---------------------------------------------------------------- host API
def make_in_maps(x, W, a_src, a_dst, ep_w, ep_b, bias, per_core):
    x = np.asarray(x, dtype=np.float32)
    W = np.asarray(W, dtype=np.float32)
    a_src = np.asarray(a_src, dtype=np.float32)
    a_dst = np.asarray(a_dst, dtype=np.float32)
    ep_w = np.asarray(ep_w, dtype=np.float32)
    ep_b = np.asarray(ep_b, dtype=np.float32)
    bias = np.asarray(bias, dtype=np.float32)

    x_pad = np.zeros((NPAD, IN_F), dtype=np.float32)
    x_pad[:N] = x
    # W [H, IN, HD] -> [IN, H*HD]
    w_flat = np.ascontiguousarray(W.transpose(1, 0, 2).reshape(IN_F, H * HD))
    as_flat = a_src.reshape(H * HD).astype(np.float32)
    ad_flat = a_dst.reshape(H * HD).astype(np.float32)

    rep = lambda v: np.ascontiguousarray(
        np.broadcast_to(v[None, :], (P, v.shape[0])))
    iota = np.broadcast_to(np.arange(P, dtype=np.float32)[None, :], (P, P))

    maps = []
    for c in range(NCORES):
        pc = per_core[c]
        x_t = np.ascontiguousarray(x_pad[c * NPC: (c + 1) * NPC, :].T)
        maps.append({
            "x_t": x_t,
            "w_in": w_flat,
            "asrep": rep(as_flat),
            "adrep": rep(ad_flat),
            "epwrep": rep(ep_w),
            "epbrep": rep(ep_b),
            "biasrep": rep(bias),
            "iotarep": np.ascontiguousarray(iota),
            "dstl_in": pc["dstl"],
            "ew_in": pc["ew"],
            "srcidx_in": pc["src_idx"],
            "dstidx_in": pc["dst_idx"],
        })
    return maps


_CACHE = {}


def kernel(x, edge_index, edge_weight, W, a_src, a_dst, ep_w, ep_b, bias):
    import hashlib
    key = hashlib.sha1(
        np.ascontiguousarray(np.asarray(edge_index, dtype=np.int64))
    ).hexdigest()
    if key not in _CACHE:
        plan, per_core = plan_and_inputs(edge_index, edge_weight)
        nc = build(plan)
        _CACHE[key] = (plan, per_core, nc)
    plan, per_core, nc = _CACHE[key]

    in_maps = make_in_maps(x, W, a_src, a_dst, ep_w, ep_b, bias, per_core)
    res = run_bass_kernel_spmd(nc, in_maps, core_ids=list(range(NCORES)),
                               trace=False)
    out_full = np.empty((NPAD, OUT_F), dtype=np.float32)
    for c in range(NCORES):
        out_full[c * NPC: (c + 1) * NPC] = res.results[c]["out"]
    return out_full[:N]



# revision 8
# speedup vs baseline: 1.6908x; 1.2261x over previous
"""EnhancedCorrelationGNN Trainium2 kernel v2 (8 NeuronCores, SPMD).

Strategy (v2 — bf16 data plane, single 512B gather, on-chip ad):
 - Host: counting-sort edges by dst, partition nodes into 8 ranges of 6272
   (49 blocks x 128 per core). Node table is split into two column groups
   A (blocks 0-23 of each core) and B (blocks 24-48) so the two AllGathers
   pipeline with phase-1 compute and phase-2 gathers. Edges are split per
   (block, table-half of src) and padded to 128-edge tiles with
   cross-core-uniform tile counts (one SPMD program).
 - Phase 1: h = x @ W plus both attention projections in ONE fp32 matmul
   per 128-node tile (rhs = [W@a_dst | W | W@a_st]); rows stored bf16 as
   [h(128)|as(8)|pad] = 512B; ad stays in SBUF (never leaves the core).
   AllGather A overlaps phase-1 B compute; AllGather B overlaps stream-A
   gathers.
 - Phase 2: per 16-tile chunk, ONE dma_gather of 512B bf16 rows by src.
   One-hot S via is_equal(dstl, iota) in bf16; S^T via PE transpose;
   ad[dst] per edge via S^T @ ad_blk matmul (no dst gather!); DVE scores
   (leaky-relu) + ScalarE exp; messages = h * p in bf16; per-tile bf16
   TensorE matmul scatter-accumulates [msgs | p] into the block PSUM;
   per block normalize by 1/(sum p + eps), add bias, DMA out.
 - No AllReduce: softmax denominators stay core-local because output is
   sharded by destination node range.
"""
import sys

if "/opt/trn_rl_repo" not in sys.path:
    sys.path.insert(0, "/opt/trn_rl_repo")

import numpy as np
from ml_dtypes import bfloat16

import concourse.bass as bass
import concourse.bacc as bacc
import concourse.mybir as mybir
import concourse.tile as tile
from concourse.bass_utils import run_bass_kernel_spmd

# ---------------------------------------------------------------- constants
N = 50000
E = 800000
IN_F = 128
H = 8
HD = 16
OUT_F = H * HD          # 128
ALPHA = 0.2
EPS = 1e-10

NCORES = 8
P = 128
NPC = 6272              # nodes per core = 49 * 128; 8*6272 = 50176 >= N
NPAD = NCORES * NPC     # 50176
NBLK = NPC // P         # 49
BLKA = 24               # blocks 0..23 -> table A
BLKB = NBLK - BLKA      # blocks 24..48 -> table B
NA = BLKA * P           # 3072 rows/core in table A
NB = BLKB * P           # 3200 rows/core in table B

ROWB = 256              # bf16 elems per table row: h(128)|as(8)|pad -> 512B
AS_OFF = 128            # attn_s offset within row
CHUNK_TILES = 16        # tiles per DVE/PE pipeline chunk
GATHER_TILES = 2        # tiles per dma_gather (divisor of CHUNK_TILES;
                        # smaller => more concurrent transfers in flight)
GP_BUFS = 8             # gather buffer depth
AGB_PIN = 11            # stream-A sub-gather index AG_B is pinned behind
                        # (negative => fraction of len: -60 means 60%)
PAD_DSTL = 300.0        # one-hot miss sentinel (exact in bf16)


def _idx_cols():
    return CHUNK_TILES * P // 16

FP = mybir.dt.float32
BF = mybir.dt.bfloat16


# ---------------------------------------------------------------- planning
def _cdiv(a, b):
    return -(-a // b)


def _wrap_idx(idx_flat: np.ndarray) -> np.ndarray:
    """[n] -> [128, IDX_COLS] int16: idx j at [j%16, j//16], replicated x8."""
    n = idx_flat.shape[0]
    assert n % 16 == 0
    w = idx_flat.reshape(n // 16, 16).T.astype(np.int16)      # [16, n/16]
    w = np.tile(w, (8, 1))                                    # [128, n/16]
    out = np.zeros((P, _idx_cols()), dtype=np.int16)
    out[:, : w.shape[1]] = w
    return out


def plan_and_inputs(edge_index, edge_weight):
    """Host-side edge partitioning. Returns (plan, per_core_arrays).

    plan (core-independent, defines the SPMD program):
      KA, KB: [NBLK] tiles per (block, stream)
      chunks: list of dicts(stream, g0, nt) over stream-major tile ids
      block_tiles: per block, list of (chunk_id, slot) in matmul order
      tile_block: per chunk, list mapping slot -> block id
      T, n_chunks
    per_core_arrays[c]:
      src_idx [n_chunks,128,IDX_COLS] i16 (table-relative)
      dstl    [128, T] bf16 (block-relative dst, PAD_DSTL for pad slots)
      ew      [128, T] f32
    """
    src = np.asarray(edge_index[0], dtype=np.int64)
    dst = np.asarray(edge_index[1], dtype=np.int64)
    ew = np.asarray(edge_weight, dtype=np.float32)

    order = np.argsort(dst, kind="stable")
    src_s, dst_s, ew_s = src[order], dst[order], ew[order]

    # stream split by src's position within its core's node range
    src_loc = src_s % NPC
    in_a = src_loc < NA
    # table-relative row index of each edge's src
    src_core = src_s // NPC
    idx_a = src_core * NA + src_loc
    idx_b = src_core * NB + (src_loc - NA)
    src_tab = np.where(in_a, idx_a, idx_b)

    cnt = np.zeros((NCORES, NBLK, 2), dtype=np.int64)
    lists = [[[None, None] for _ in range(NBLK)] for _ in range(NCORES)]
    blk_starts = np.searchsorted(dst_s, np.arange(0, NPAD + 1, P))
    for c in range(NCORES):
        for b in range(NBLK):
            g = c * NBLK + b
            lo, hi = blk_starts[g], blk_starts[g + 1]
            mA = in_a[lo:hi]
            idxs = np.arange(lo, hi)
            lists[c][b][0] = idxs[mA]
            lists[c][b][1] = idxs[~mA]
            cnt[c, b, 0] = mA.sum()
            cnt[c, b, 1] = (~mA).sum()

    KA = np.maximum(_cdiv(cnt[:, :, 0].max(axis=0), P), 1).astype(np.int64)
    KB = _cdiv(cnt[:, :, 1].max(axis=0), P).astype(np.int64)

    T_A = int(KA.sum())
    T_B = int(KB.sum())
    T = T_A + T_B
    cumKA = np.concatenate([[0], np.cumsum(KA)])
    cumKB = np.concatenate([[0], np.cumsum(KB)])

    # chunks: stream-major [0,T_A) then [T_A,T)
    chunks = []
    g = 0
    while g < T_A:
        nt = min(CHUNK_TILES, T_A - g)
        chunks.append(dict(stream=0, g0=g, nt=nt))
        g += nt
    while g < T:
        nt = min(CHUNK_TILES, T - g)
        chunks.append(dict(stream=1, g0=g, nt=nt))
        g += nt
    n_chunks = len(chunks)

    chunk_of = np.empty(T, dtype=np.int64)
    slot_of = np.empty(T, dtype=np.int64)
    for ci, ch in enumerate(chunks):
        chunk_of[ch["g0"]: ch["g0"] + ch["nt"]] = ci
        slot_of[ch["g0"]: ch["g0"] + ch["nt"]] = np.arange(ch["nt"])

    block_tiles = []
    tile_block = [[None] * ch["nt"] for ch in chunks]
    for b in range(NBLK):
        tl = []
        for k in range(KA[b]):
            gidx = cumKA[b] + k
            tl.append((int(chunk_of[gidx]), int(slot_of[gidx])))
        for k in range(KB[b]):
            gidx = T_A + cumKB[b] + k
            tl.append((int(chunk_of[gidx]), int(slot_of[gidx])))
        block_tiles.append(tl)
        for (ci, slot) in tl:
            tile_block[ci][slot] = b
    assert all(b is not None for tb in tile_block for b in tb)

    plan = dict(KA=KA, KB=KB, T=T, T_A=T_A, chunks=chunks,
                block_tiles=block_tiles, tile_block=tile_block,
                n_chunks=n_chunks)

    # ---------------- per-core slot arrays
    per_core = []
    for c in range(NCORES):
        src_rel = np.zeros((T, P), dtype=np.int16)
        dstl = np.full((T, P), PAD_DSTL, dtype=np.float32)
        eww = np.zeros((T, P), dtype=np.float32)
        for b in range(NBLK):
            for half, K, cum, base in ((0, KA, cumKA, 0),
                                       (1, KB, cumKB, T_A)):
                idxs = lists[c][b][half]
                n = idxs.shape[0]
                g0 = base + cum[b]
                nslots = int(K[b]) * P
                s_loc = np.zeros(nslots, dtype=np.int64)
                dl = np.full(nslots, PAD_DSTL, dtype=np.float32)
                w = np.zeros(nslots, dtype=np.float32)
                if n:
                    s_loc[:n] = src_tab[idxs]
                    dl[:n] = (dst_s[idxs] - (c * NPC + b * P)).astype(
                        np.float32)
                    w[:n] = ew_s[idxs]
                src_rel[g0: g0 + int(K[b])] = s_loc.reshape(
                    int(K[b]), P).astype(np.int16)
                dstl[g0: g0 + int(K[b])] = dl.reshape(int(K[b]), P)
                eww[g0: g0 + int(K[b])] = w.reshape(int(K[b]), P)

        src_idx = np.zeros((n_chunks, P, _idx_cols()), dtype=np.int16)
        for ci, ch in enumerate(chunks):
            g0, nt = ch["g0"], ch["nt"]
            src_idx[ci] = _wrap_idx(src_rel[g0: g0 + nt].reshape(nt * P))

        per_core.append(dict(
            src_idx=src_idx,
            dstl=np.ascontiguousarray(dstl.T).astype(bfloat16),  # [128, T]
            ew=np.ascontiguousarray(eww.T),                      # [128, T]
        ))

    return plan, per_core


# build stages for HW bisection: 1=phase1+AGs only, 2=+gathers,
# 3=+chunk pipeline (S/transpose/ad/scores/msgs), 4=full (default)
BUILD_STAGE = 4
# repeat whole kernel body inside one NEFF (for timing by differencing)
REPS = 1


# ---------------------------------------------------------------- builder
def build(plan):
    n_chunks = plan["n_chunks"]
    chunks = plan["chunks"]
    tile_block = plan["tile_block"]
    T = plan["T"]

    nc = bacc.Bacc("TRN2", target_bir_lowering=False, debug=False,
                   num_devices=NCORES, num_swdge_queues=4)
    qctr = [0]

    # inputs
    x_t = nc.dram_tensor("x_t", [P, NPC], BF, kind="ExternalInput")
    w_in = nc.dram_tensor("w_in", [P, IN_F], FP, kind="ExternalInput")
    asrep = nc.dram_tensor("asrep", [P, IN_F], FP, kind="ExternalInput")
    adrep = nc.dram_tensor("adrep", [P, IN_F], FP, kind="ExternalInput")
    epwrep = nc.dram_tensor("epwrep", [P, H], FP, kind="ExternalInput")
    epbrep = nc.dram_tensor("epbrep", [P, H], FP, kind="ExternalInput")
    biasrep = nc.dram_tensor("biasrep", [P, OUT_F], FP, kind="ExternalInput")
    iotarep = nc.dram_tensor("iotarep", [P, P], BF, kind="ExternalInput")
    identrep = nc.dram_tensor("identrep", [P, P], BF, kind="ExternalInput")
    dstl_in = nc.dram_tensor("dstl_in", [P, T], BF, kind="ExternalInput")
    ew_in = nc.dram_tensor("ew_in", [P, T], FP, kind="ExternalInput")
    srcidx_in = nc.dram_tensor("srcidx_in", [n_chunks, P, _idx_cols()],
                               mybir.dt.int16, kind="ExternalInput")
    out = nc.dram_tensor("out", [NPC, OUT_F], FP, kind="ExternalOutput")

    with tile.TileContext(nc) as tc:
        for _rep in range(REPS):
            with tc.tile_pool(name="dram", bufs=1, space="DRAM") as dram, \
                 tc.tile_pool(name="statics", bufs=1) as statics:

                hs_inA = dram.tile([NA, ROWB], BF)
                hs_fullA = dram.tile([NCORES * NA, ROWB], BF,
                                     addr_space="Shared")
                hs_inB = dram.tile([NB, ROWB], BF)
                hs_fullB = dram.tile([NCORES * NB, ROWB], BF,
                                     addr_space="Shared")

                # ---------------- statics
                iota_sb = statics.tile([P, P], BF)
                nc.sync.dma_start(iota_sb[:], iotarep[:])
                ident_sb = statics.tile([P, P], BF)
                nc.sync.dma_start(ident_sb[:], identrep[:])
                epw_sb = statics.tile([P, H], FP)
                nc.sync.dma_start(epw_sb[:], epwrep[:])
                epb_sb = statics.tile([P, H], FP)
                nc.sync.dma_start(epb_sb[:], epbrep[:])
                bias_sb = statics.tile([P, OUT_F], FP)
                nc.sync.dma_start(bias_sb[:], biasrep[:])
                ad_all = statics.tile([P, NBLK * H], BF)

                # ---------------- phase 1: node table
                with tc.tile_pool(name="p1", bufs=1) as p1, \
                     tc.tile_pool(name="p1psum", bufs=4, space="PSUM") as p1ps:
                    w_sb = p1.tile([P, IN_F], FP)
                    nc.sync.dma_start(w_sb[:], w_in[:])
                    as_sb = p1.tile([P, IN_F], FP)
                    nc.sync.dma_start(as_sb[:], asrep[:])
                    ad_sb = p1.tile([P, IN_F], FP)
                    nc.sync.dma_start(ad_sb[:], adrep[:])
                    xt_sb = p1.tile([P, NPC], BF)
                    nc.sync.dma_start(xt_sb[:], x_t[:])

                    # rhs_w = [W@a_dst | W | W@a_src]  [128, 144] bf16
                    rhs_w = p1.tile([P, IN_F + 2 * H], BF)
                    nc.vector.tensor_copy(rhs_w[:, H: H + IN_F], w_sb[:])
                    tmp_d = p1.tile([P, IN_F], FP)
                    nc.vector.tensor_tensor(out=tmp_d[:], in0=w_sb[:],
                                            in1=ad_sb[:],
                                            op=mybir.AluOpType.mult)
                    tmp_s = p1.tile([P, IN_F], FP)
                    nc.vector.tensor_tensor(out=tmp_s[:], in0=w_sb[:],
                                            in1=as_sb[:],
                                            op=mybir.AluOpType.mult)
                    with nc.allow_low_precision(
                            reason="bf16 table rows; tolerance 2e-2"):
                        nc.vector.tensor_reduce(
                            out=rhs_w[:, 0:H],
                            in_=tmp_d[:].rearrange("p (h d) -> p h d", d=HD),
                            axis=mybir.AxisListType.X,
                            op=mybir.AluOpType.add)
                        nc.vector.tensor_reduce(
                            out=rhs_w[:, H + IN_F: H + IN_F + H],
                            in_=tmp_s[:].rearrange("p (h d) -> p h d", d=HD),
                            axis=mybir.AxisListType.X,
                            op=mybir.AluOpType.add)

                    hs_sliceA = p1.tile([P, BLKA * ROWB], BF)
                    hs_sliceB = p1.tile([P, BLKB * ROWB], BF)
                    # zero pad columns (DMA'd/gathered but never read)
                    nc.vector.memset(
                        hs_sliceA[:].rearrange("p (t r) -> p t r", r=ROWB)
                        [:, :, AS_OFF + H: ROWB], 0.0)
                    nc.vector.memset(
                        hs_sliceB[:].rearrange("p (t r) -> p t r", r=ROWB)
                        [:, :, AS_OFF + H: ROWB], 0.0)

                    def do_block(t):
                        hpsum = p1ps.tile([P, IN_F + 2 * H], FP, space="PSUM")
                        nc.tensor.matmul(out=hpsum[:],
                                         lhsT=xt_sb[:, t * P: (t + 1) * P],
                                         rhs=rhs_w[:], start=True, stop=True)
                        # [ad | h | as] -> slice gets [h|as] bf16, ad_all bf16
                        if t < BLKA:
                            sl, tt = hs_sliceA, t
                        else:
                            sl, tt = hs_sliceB, t - BLKA
                        nc.vector.tensor_copy(
                            sl[:, tt * ROWB: tt * ROWB + IN_F + H],
                            hpsum[:, H: 2 * H + IN_F])
                        nc.scalar.activation(
                            ad_all[:, t * H: (t + 1) * H],
                            hpsum[:, 0:H],
                            mybir.ActivationFunctionType.Copy)

                    for t in range(BLKA):
                        do_block(t)
                    nc.sync.dma_start(
                        hs_inA[:].rearrange("(t p) r -> p t r", p=P),
                        hs_sliceA[:].rearrange("p (t r) -> p t r", r=ROWB))
                    nc.gpsimd.collective_compute(
                        "AllGather", mybir.AluOpType.bypass,
                        replica_groups=[list(range(NCORES))],
                        ins=[hs_inA[:]], outs=[hs_fullA[:]],
                    )
                    for t in range(BLKA, NBLK):
                        do_block(t)
                    nc.sync.dma_start(
                        hs_inB[:].rearrange("(t p) r -> p t r", p=P),
                        hs_sliceB[:].rearrange("p (t r) -> p t r", r=ROWB))
                    # AG_B is emitted lazily in phase 2 (after the stream-A
                    # gather instructions) so its Pool-blocking transfer
                    # overlaps the async stream-A gather DMA.

                # ---------------- phase 2
                with tc.tile_pool(name="meta", bufs=1) as meta, \
                     tc.tile_pool(name="gp", bufs=GP_BUFS) as gp, \
                     tc.tile_pool(name="sp", bufs=4) as sp, \
                     tc.tile_pool(name="stp", bufs=4) as stp, \
                     tc.tile_pool(name="rp", bufs=4) as rp, \
                     tc.tile_pool(name="ep", bufs=3) as ep, \
                     tc.tile_pool(name="op", bufs=3) as opool, \
                     tc.tile_pool(name="tp", bufs=2, space="PSUM") as tp, \
                     tc.tile_pool(name="adps", bufs=2, space="PSUM") as adps, \
                     tc.tile_pool(name="bps", bufs=4, space="PSUM") as bps:

                    dstl_sb = meta.tile([P, T], BF)
                    nc.sync.dma_start(dstl_sb[:], dstl_in[:])
                    ew_sb = meta.tile([P, T], FP)
                    nc.sync.dma_start(ew_sb[:], ew_in[:])
                    sidx_all = meta.tile([P, n_chunks, _idx_cols()],
                                         mybir.dt.int16)
                    nc.sync.dma_start(
                        sidx_all[:],
                        srcidx_in[:].rearrange("c p i -> p c i"))
                    # per-block stream-A accumulator (lets all stream-A work
                    # finish before any stream-B gather is issued, so the
                    # B AllGather overlaps stream-A gathers)
                    accA = meta.tile([P, NBLK * (OUT_F + H)], FP)

                    chunk_tiles = {}
                    agb_done = [False]
                    gather_insts = []

                    def ensure_agb():
                        if not agb_done[0]:
                            agb_done[0] = True
                            cc = nc.gpsimd.collective_compute(
                                "AllGather", mybir.AluOpType.bypass,
                                replica_groups=[list(range(NCORES))],
                                ins=[hs_inB[:]], outs=[hs_fullB[:]],
                            )
                            # pin AG_B behind an early stream-A gather so the
                            # scheduler can't hoist it ahead of them (its
                            # Pool-blocking transfer then overlaps the async
                            # stream-A gather DMA)
                            if gather_insts:
                                from concourse.tile import add_dep_helper
                                if AGB_PIN < 0:
                                    pin = (len(gather_insts) * -AGB_PIN) // 100
                                else:
                                    pin = AGB_PIN
                                g = gather_insts[min(pin,
                                                     len(gather_insts) - 1)]
                                add_dep_helper(
                                    cc.ins, g.ins,
                                    reason="overlap AG_B with stream-A "
                                           "gathers")

                    def emit_chunk(ci):
                        ch = chunks[ci]
                        g0, nt = ch["g0"], ch["nt"]
                        nidx = nt * P
                        n16 = nidx // 16
                        if BUILD_STAGE == 1:
                            return
                        if ch["stream"] == 1:
                            ensure_agb()

                        sidx = sidx_all[:, ci, :]
                        gbuf = gp.tile([P, CHUNK_TILES, ROWB], BF, tag="gbuf")
                        tab = hs_fullA if ch["stream"] == 0 else hs_fullB
                        # split the chunk's gather into GATHER_TILES-sized
                        # sub-gathers so many transfers are in flight at once
                        for g in range(0, nt, GATHER_TILES):
                            gn = min(GATHER_TILES, nt - g)
                            gi = nc.gpsimd.dma_gather(
                                out_ap=gbuf[:, g: g + gn, :], in_ap=tab[:],
                                idxs_ap=sidx[:, g * 8: (g + gn) * 8],
                                num_idxs=gn * P, num_idxs_reg=gn * P,
                                elem_size=ROWB,
                                single_packet=False, queue_num=qctr[0] % 4)
                            if ch["stream"] == 0:
                                gather_insts.append(gi)
                            qctr[0] += 1
                        if BUILD_STAGE == 2:
                            chunk_tiles[ci] = (gbuf, None)
                            return

                        # one-hot S [P, nt, 128] bf16
                        s_t = sp.tile([P, CHUNK_TILES * P], BF, tag="s_t")
                        s_v = s_t[:].rearrange("p (t n) -> p t n", n=P)
                        dstl_v = dstl_sb[:, g0: g0 + nt]
                        nc.vector.tensor_tensor(
                            out=s_v[:, :nt, :],
                            in0=dstl_v.unsqueeze(2).broadcast_to([P, nt, P]),
                            in1=iota_sb[:].unsqueeze(1).broadcast_to(
                                [P, nt, P]),
                            op=mybir.AluOpType.is_equal)

                        # S^T per tile via PE transpose; 8 tiles per PSUM bank
                        # (PSUM->SBUF drain on the otherwise-idle ScalarE)
                        st_sb = stp.tile([P, CHUNK_TILES * P], BF, tag="st")
                        for grp in range(0, nt, 8):
                            gn = min(8, nt - grp)
                            ps_t = tp.tile([P, 8 * P], BF, space="PSUM",
                                           tag="ps_t")
                            for k in range(gn):
                                nc.tensor.transpose(
                                    ps_t[:, k * P: (k + 1) * P],
                                    s_t[:, (grp + k) * P: (grp + k + 1) * P],
                                    ident_sb[:])
                            nc.scalar.activation(
                                st_sb[:, grp * P: (grp + gn) * P],
                                ps_t[:, 0: gn * P],
                                mybir.ActivationFunctionType.Copy)

                        # ad[dst] per edge: S^T @ ad_blk  [P, nt*H] fp32 psum
                        ps_ad = adps.tile([P, CHUNK_TILES * H], FP,
                                          space="PSUM", tag="ps_ad")
                        for slot in range(nt):
                            b = tile_block[ci][slot]
                            nc.tensor.matmul(
                                out=ps_ad[:, slot * H: (slot + 1) * H],
                                lhsT=st_sb[:, slot * P: (slot + 1) * P],
                                rhs=ad_all[:, b * H: (b + 1) * H],
                                start=True, stop=True)

                        # scores: e = leaky(as+ad) + ew*epw + epb; p = exp(e)
                        ad_v = ps_ad[:].rearrange("p (t h) -> p t h",
                                                  h=H)[:, :nt, :]
                        e0 = ep.tile([P, CHUNK_TILES * H], FP, tag="e0")
                        e0v = e0[:].rearrange("p (t h) -> p t h",
                                              h=H)[:, :nt, :]
                        nc.vector.tensor_tensor(
                            out=e0v, in0=gbuf[:, :nt, AS_OFF: AS_OFF + H],
                            in1=ad_v, op=mybir.AluOpType.add)
                        # leaky-relu in one op: e2 = max(alpha*e0, e0)
                        e2 = ep.tile([P, CHUNK_TILES * H], FP, tag="e2")
                        e2v = e2[:].rearrange("p (t h) -> p t h",
                                              h=H)[:, :nt, :]
                        nc.vector.scalar_tensor_tensor(
                            out=e2v, in0=e0v, scalar=ALPHA, in1=e0v,
                            op0=mybir.AluOpType.mult,
                            op1=mybir.AluOpType.max)
                        # c = ew*epw + epb (gather-independent)
                        c1 = ep.tile([P, CHUNK_TILES * H], FP, tag="c1")
                        c1v = c1[:].rearrange("p (t h) -> p t h",
                                              h=H)[:, :nt, :]
                        nc.vector.tensor_tensor(
                            out=c1v,
                            in0=ew_sb[:, g0: g0 + nt].unsqueeze(2)
                                .broadcast_to([P, nt, H]),
                            in1=epw_sb[:].unsqueeze(1).broadcast_to(
                                [P, nt, H]),
                            op=mybir.AluOpType.mult)
                        c2 = ep.tile([P, CHUNK_TILES * H], FP, tag="c2")
                        c2v = c2[:].rearrange("p (t h) -> p t h",
                                              h=H)[:, :nt, :]
                        nc.vector.tensor_tensor(out=c2v, in0=c1v,
                                                in1=epb_sb[:].unsqueeze(1)
                                                .broadcast_to([P, nt, H]),
                                                op=mybir.AluOpType.add)
                        e5 = ep.tile([P, CHUNK_TILES * H], FP, tag="e5")
                        e5v = e5[:].rearrange("p (t h) -> p t h",
                                              h=H)[:, :nt, :]
                        nc.vector.tensor_tensor(out=e5v, in0=e2v, in1=c2v,
                                                op=mybir.AluOpType.add)

                        # rhs tile: [msgs(128) | p(8)] per tile, bf16
                        rhs = rp.tile([P, CHUNK_TILES * (OUT_F + H)], BF,
                                      tag="rhs")
                        rhs_v = rhs[:].rearrange("p (t f) -> p t f",
                                                 f=OUT_F + H)
                        nc.scalar.activation(
                            rhs_v[:, :nt, OUT_F: OUT_F + H], e5v,
                            mybir.ActivationFunctionType.Exp)
                        nc.vector.tensor_tensor(
                            out=rhs_v[:, :nt, 0:OUT_F].rearrange(
                                "p t (h d) -> p t h d", d=HD),
                            in0=gbuf[:, :nt, 0:IN_F].rearrange(
                                "p t (h d) -> p t h d", d=HD),
                            in1=rhs_v[:, :nt, OUT_F: OUT_F + H].unsqueeze(3)
                                .broadcast_to([P, nt, H, HD]),
                            op=mybir.AluOpType.mult)
                        chunk_tiles[ci] = (s_t, rhs)

                    if BUILD_STAGE < 4:
                        for ci in range(n_chunks):
                            emit_chunk(ci)
                        ensure_agb()
                        dump = opool.tile([P, OUT_F], FP, tag="dump")
                        if BUILD_STAGE == 1:
                            nc.vector.memset(dump[:], 0.0)
                        elif BUILD_STAGE == 2:
                            g0buf = chunk_tiles[0][0]
                            nc.vector.tensor_copy(dump[:],
                                                  g0buf[:, 0, 0:OUT_F])
                        else:
                            r0 = chunk_tiles[0][1]
                            nc.vector.tensor_copy(dump[:], r0[:, 0:OUT_F])
                        for b in range(NBLK):
                            nc.sync.dma_start(out[b * P: (b + 1) * P, :],
                                              dump[:])

                    def accum(tl, start_fresh=True):
                        psum_b = bps.tile([P, OUT_F + H], FP, space="PSUM",
                                          tag="psum_b")
                        for i, (ci, slot) in enumerate(tl):
                            s_t, rhs = chunk_tiles[ci]
                            nc.tensor.matmul(
                                out=psum_b[:],
                                lhsT=s_t[:, slot * P: (slot + 1) * P],
                                rhs=rhs[:, slot * (OUT_F + H):
                                        (slot + 1) * (OUT_F + H)],
                                start=(i == 0), stop=(i == len(tl) - 1))
                        return psum_b

                    if BUILD_STAGE >= 4:
                        KA = plan["KA"]
                        # stream A pass: accumulate into accA (via ScalarE)
                        for b in range(NBLK):
                            tlA = plan["block_tiles"][b][: int(KA[b])]
                            for (ci, slot) in tlA:
                                if ci not in chunk_tiles:
                                    emit_chunk(ci)
                            psum_b = accum(tlA)
                            nc.scalar.activation(
                                accA[:, b * (OUT_F + H):
                                     (b + 1) * (OUT_F + H)],
                                psum_b[:],
                                mybir.ActivationFunctionType.Copy)
                        # stream B pass: accumulate, combine, normalize, out
                        for b in range(NBLK):
                            tlB = plan["block_tiles"][b][int(KA[b]):]
                            for (ci, slot) in tlB:
                                if ci not in chunk_tiles:
                                    emit_chunk(ci)
                            accv = accA[:, b * (OUT_F + H):
                                        (b + 1) * (OUT_F + H)]
                            if tlB:
                                psum_b = accum(tlB)
                                comb = opool.tile([P, OUT_F + H], FP,
                                                  tag="comb")
                                nc.vector.tensor_tensor(
                                    out=comb[:], in0=psum_b[:], in1=accv,
                                    op=mybir.AluOpType.add)
                                srcv = comb[:]
                            else:
                                srcv = accv
                            # normalize + bias
                            s_eps = opool.tile([P, H], FP, tag="s_eps")
                            nc.vector.tensor_scalar_add(
                                out=s_eps[:], in0=srcv[:, OUT_F: OUT_F + H],
                                scalar1=EPS)
                            rcp = opool.tile([P, H], FP, tag="rcp")
                            nc.vector.reciprocal(rcp[:], s_eps[:])
                            ob1 = opool.tile([P, OUT_F], FP, tag="ob1")
                            nc.vector.tensor_tensor(
                                out=ob1[:].rearrange("p (h d) -> p h d",
                                                     d=HD),
                                in0=srcv[:, 0:OUT_F].rearrange(
                                    "p (h d) -> p h d", d=HD),
                                in1=rcp[:].unsqueeze(2).broadcast_to(
                                    [P, H, HD]),
                                op=mybir.AluOpType.mult)
                            ob2 = opool.tile([P, OUT_F], FP, tag="ob2")
                            nc.vector.tensor_tensor(out=ob2[:], in0=ob1[:],
                                                    in1=bias_sb[:],
                                                    op=mybir.AluOpType.add)
                            nc.sync.dma_start(out[b * P: (b + 1) * P, :],
                                              ob2[:])

    nc.compile()
    # SWDGE constraint: a DMA semaphore may only be updated from one queue.
    # Tile assigns DMASW lanes post-scheduling, so align queue_num to lane.
    for f in nc.m.functions:
        for bb in f.blocks:
            for ins in bb.instructions:
                if type(ins).__name__ == "InstDMAGatherAnt":
                    si = ins.sync_info
                    lane = None
                    for u in si.on_update:
                        nm = u.ant_name or ""
                        if nm.startswith("DMASW"):
                            lane = int(nm[5:].split("_")[0])
                            break
                    assert lane is not None, "gather without DMASW sem"
                    ins.queue_num = lane % 4
    return nc


# ---------------------------------------------------------------- host API
def make_in_maps(x, W, a_src, a_dst, ep_w, ep_b, bias, per_core):
    x = np.asarray(x, dtype=np.float32)
    W = np.asarray(W, dtype=np.float32)
    a_src = np.asarray(a_src, dtype=np.float32)
    a_dst = np.asarray(a_dst, dtype=np.float32)
    ep_w = np.asarray(ep_w, dtype=np.float32)
    ep_b = np.asarray(ep_b, dtype=np.float32)
    bias = np.asarray(bias, dtype=np.float32)

    x_pad = np.zeros((NPAD, IN_F), dtype=np.float32)
    x_pad[:N] = x
    # W [H, IN, HD] -> [IN, H*HD]
    w_flat = np.ascontiguousarray(W.transpose(1, 0, 2).reshape(IN_F, H * HD))
    as_flat = a_src.reshape(H * HD).astype(np.float32)
    ad_flat = a_dst.reshape(H * HD).astype(np.float32)

    rep = lambda v: np.ascontiguousarray(
        np.broadcast_to(v[None, :], (P, v.shape[0])))
    iota = np.broadcast_to(
        np.arange(P, dtype=np.float32)[None, :], (P, P)).astype(bfloat16)
    ident = np.eye(P, dtype=np.float32).astype(bfloat16)

    maps = []
    for c in range(NCORES):
        pc = per_core[c]
        x_t = np.ascontiguousarray(
            x_pad[c * NPC: (c + 1) * NPC, :].T).astype(bfloat16)
        maps.append({
            "x_t": x_t,
            "w_in": w_flat,
            "asrep": rep(as_flat),
            "adrep": rep(ad_flat),
            "epwrep": rep(ep_w),
            "epbrep": rep(ep_b),
            "biasrep": rep(bias),
            "iotarep": np.ascontiguousarray(iota),
            "identrep": np.ascontiguousarray(ident),
            "dstl_in": pc["dstl"],
            "ew_in": pc["ew"],
            "srcidx_in": pc["src_idx"],
        })
    return maps


_CACHE = {}


def kernel(x, edge_index, edge_weight, W, a_src, a_dst, ep_w, ep_b, bias):
    import hashlib
    key = hashlib.sha1(
        np.ascontiguousarray(np.asarray(edge_index, dtype=np.int64))
    ).hexdigest()
    if key not in _CACHE:
        plan, per_core = plan_and_inputs(edge_index, edge_weight)
        nc = build(plan)
        _CACHE[key] = (plan, per_core, nc)
    plan, per_core, nc = _CACHE[key]

    in_maps = make_in_maps(x, W, a_src, a_dst, ep_w, ep_b, bias, per_core)
    res = run_bass_kernel_spmd(nc, in_maps, core_ids=list(range(NCORES)),
                               trace=False)
    out_full = np.empty((NPAD, OUT_F), dtype=np.float32)
    for c in range(NCORES):
        out_full[c * NPC: (c + 1) * NPC] = res.results[c]["out"]
    return out_full[:N]
